# revision 59
# baseline (speedup 1.0000x reference)
"""DeepSeekMoE kernel for 8 TRN2 NeuronCores.

Sharding: load-balanced expert-parallel. Each routed expert's FFN is split
in half along the hidden (H) axis across two cores, and the 4 heaviest
experts (by routed-token count) are paired with the 4 lightest, so every
core carries one heavy half-expert (slot A) and one light half-expert
(slot B) — per-core matmul rows become nearly uniform instead of every
core paying the max expert's padding. Each core also owns a 1/8 H-shard
of the shared expert (tensor-parallel).

The tiny gate (sigmoid + top-2 over E=8) runs on host; tokens are gathered
per expert, padded to per-slot caps (SPMD: one program for all 8 cores),
and shipped pre-transposed so every device-side matmul contracts over the
partition dimension. Each core returns
  yea/yeb: [D, capA/B] half-expert outputs, scaled by the combine weight
  sh:      [T, D]      shared-expert partial (its H-shard, bf16)
Host scatters ye back by token index (the two halves of an expert sum via
the scatter-add) and sums the 8 sh partials — the output gather performs
the MoE combine; no on-device collectives needed.

Device kernel (per core, TensorE-bound):
  warmup (flips the HAM clock gate to 2.4GHz during the initial DMAs)
  B:  hT = gelu(W1half^T x_e)   for slot A then slot B; streamed pair-slabs
  C:  ye = w * (W2half^T hT)    streamed d-slabs, tokens as the moving dim
  D:  hsT = gelu(Ws1^T x)       all T tokens through this core's H-shard
  E:  sh = Ws2^T hsT
DMA emission follows usage order: all DGE paths share the 16 SDMA engines,
so later-phase weights stream behind the slabs that gate the PE.

Compute dtype: bf16 operands, fp32 PSUM accumulation (rel err ~3.4e-3).
"""

import hashlib
import sys

sys.path.insert(0, "/opt/trn_rl_repo")

import numpy as np
import ml_dtypes

import concourse.bass as bass
import concourse.bacc as bacc
import concourse.mybir as mybir
import concourse.tile as tile
from concourse.bass_utils import run_bass_kernel_spmd

BF16 = ml_dtypes.bfloat16
F32 = np.float32

T, D, E, TOP_K, H = 2048, 1024, 8, 2, 4096
H2 = H // 2          # half-expert hidden
HS = H // 8          # shared-expert hidden shard per core
KD = D // 128        # 8  k-chunks over D
KH2 = H2 // 128      # 16 k-chunks over a half-expert
KHS = HS // 128      # 4  k-chunks over the shared shard
N_CORES = 8

_DT = mybir.dt.bfloat16
_cache: dict = {}
_wcache: dict = {}


def _tchunks(cap):
    out, s = [], 0
    while s < cap:
        out.append((s, min(512, cap - s)))
        s += 512
    return out


def _build(caps):
    """Build + finalize the SPMD device program for slot caps (capA, capB)."""
    nc = bacc.Bacc("TRN2", target_bir_lowering=False, debug=False)

    xe_d, w1_d, w2_d, b1_d, wr_d, ye_d = {}, {}, {}, {}, {}, {}
    for s, cap in zip("ab", caps):
        assert cap % 64 == 0
        xe_d[s] = nc.dram_tensor(f"xe{s}", [128, KD, cap], _DT, kind="ExternalInput")
        w1_d[s] = nc.dram_tensor(f"w1{s}", [KH2 // 2, 128, KD, 256], _DT, kind="ExternalInput")
        w2_d[s] = nc.dram_tensor(f"w2{s}", [8, 128, KH2, 128], _DT, kind="ExternalInput")
        b1_d[s] = nc.dram_tensor(f"b1{s}", [128, KH2], mybir.dt.float32, kind="ExternalInput")
        wr_d[s] = nc.dram_tensor(f"wr{s}", [128, cap], mybir.dt.float32, kind="ExternalInput")
        ye_d[s] = nc.dram_tensor(f"ye{s}", [D, cap], mybir.dt.float32, kind="ExternalOutput")
    xt_d = nc.dram_tensor("xt", [4, 128, KD, 512], _DT, kind="ExternalInput")
    ws1_d = nc.dram_tensor("ws1", [128, KD, HS], _DT, kind="ExternalInput")
    ws2_d = nc.dram_tensor("ws2", [128, KHS, D], _DT, kind="ExternalInput")
    bs1_d = nc.dram_tensor("bs1c", [128, KHS], mybir.dt.float32, kind="ExternalInput")
    sh_d = nc.dram_tensor("sh", [T, D], _DT, kind="ExternalOutput")

    gelu = mybir.ActivationFunctionType.Gelu

    with tile.TileContext(nc) as tc:
        with (
            tc.tile_pool(name="resident", bufs=1) as rpool,
            tc.tile_pool(name="w1s", bufs=4) as w1pool,
            tc.tile_pool(name="w2s", bufs=4) as w2pool,
            tc.tile_pool(name="xts", bufs=3) as xtpool,
            tc.tile_pool(name="psum", bufs=7, space="PSUM") as pspool,
            tc.tile_pool(name="wpsum", bufs=1, space="PSUM") as wpspool,
            tc.tile_pool(name="outs", bufs=6) as opool,
        ):
            # ---- PE warmup: dummy matmuls while the first DMAs are in
            # flight, so the HAM clock gate is at 2.4GHz for the real work.
            scratch = rpool.tile([128, 512], _DT)
            nc.vector.memset(scratch[:], 0.0)
            wps = wpspool.tile([128, 512], mybir.dt.float32)
            for _ in range(9):
                nc.tensor.matmul(wps[:], scratch[:, 0:128], scratch[:], start=True, stop=True)

            # ---- phase-B-critical loads, in first-consumption order ----
            w1s0 = w1pool.tile([128, KD, 256], _DT)
            nc.sync.dma_start(w1s0[:, :, 0:128], w1_d["a"][0, :, :, 0:128])
            xe_sb, wr_sb, b1_sb, hT = {}, {}, {}, {}
            xe_sb["a"] = rpool.tile([128, KD, caps[0]], _DT, name="xea", tag="xea")
            nc.sync.dma_start(xe_sb["a"][:, 0:KD // 2, :], xe_d["a"][:, 0:KD // 2, :])
            nc.sync.dma_start(xe_sb["a"][:, KD // 2:, :], xe_d["a"][:, KD // 2:, :])
            nc.sync.dma_start(w1s0[:, :, 128:256], w1_d["a"][0, :, :, 128:256])
            b1_sb["a"] = rpool.tile([128, KH2], mybir.dt.float32, name="b1a", tag="b1a")
            nc.sync.dma_start(b1_sb["a"][:], b1_d["a"][:])

            hT["a"] = rpool.tile([128, KH2, caps[0]], _DT, name="hTa", tag="hTa")
            hT["b"] = rpool.tile([128, KH2, caps[1]], _DT, name="hTb", tag="hTb")
            hsT = rpool.tile([128, KHS, T], _DT)

            # ---- phase B: routed GEMM1 per slot ----
            for si, s in enumerate("ab"):
                cap = caps[si]
                if s == "b":
                    xe_sb["b"] = rpool.tile([128, KD, cap], _DT, name="xeb", tag="xeb")
                    nc.sync.dma_start(xe_sb["b"][:], xe_d["b"][:])
                    b1_sb["b"] = rpool.tile([128, KH2], mybir.dt.float32, name="b1b", tag="b1b")
                    nc.sync.dma_start(b1_sb["b"][:], b1_d["b"][:])
                for hp in range(KH2 // 2):
                    if s == "a" and hp == 0:
                        w1s = w1s0
                    else:
                        w1s = w1pool.tile([128, KD, 256], _DT)
                        nc.sync.dma_start(w1s[:], w1_d[s][hp])
                    for hh in range(2):
                        h = 2 * hp + hh
                        for (t0, tsz) in _tchunks(cap):
                            ps = pspool.tile([128, 512], mybir.dt.float32)
                            for k in range(KD):
                                nc.tensor.matmul(
                                    ps[:, :tsz],
                                    w1s[:, k, hh * 128:hh * 128 + 128],
                                    xe_sb[s][:, k, t0:t0 + tsz],
                                    start=(k == 0),
                                    stop=(k == KD - 1),
                                )
                            nc.scalar.activation(
                                hT[s][:, h, t0:t0 + tsz], ps[:, :tsz], gelu,
                                bias=b1_sb[s][:, h:h + 1],
                            )

            # ---- phase C: routed GEMM2 (tokens moving) + weight scale ----
            ws1_sb = rpool.tile([128, KD, HS], _DT)
            ws2_sb = rpool.tile([128, KHS, D], _DT)
            bs1_sb = rpool.tile([128, KHS], mybir.dt.float32)
            for si, s in enumerate("ab"):
                cap = caps[si]
                wr_sb[s] = rpool.tile([128, cap], mybir.dt.float32, name=f"wr{s}", tag=f"wr{s}")
                nc.sync.dma_start(wr_sb[s][:], wr_d[s][:])
                for d in range(8):
                    w2s = w2pool.tile([128, KH2, 128], _DT)
                    nc.sync.dma_start(w2s[:], w2_d[s][d])
                    # shared-expert loads trickle behind the early slabs
                    if s == "a" and d == 0:
                        nc.sync.dma_start(ws1_sb[:], ws1_d[:])
                    elif s == "a" and d == 1:
                        nc.sync.dma_start(ws2_sb[:], ws2_d[:])
                    elif s == "a" and d == 2:
                        nc.sync.dma_start(bs1_sb[:], bs1_d[:])
                    for (t0, tsz) in _tchunks(cap):
                        ps = pspool.tile([128, 512], mybir.dt.float32)
                        for k in range(KH2):
                            nc.tensor.matmul(
                                ps[:, :tsz],
                                w2s[:, k, :],
                                hT[s][:, k, t0:t0 + tsz],
                                start=(k == 0),
                                stop=(k == KH2 - 1),
                            )
                        eo = opool.tile([128, 512], mybir.dt.float32, tag="eo")
                        nc.vector.tensor_mul(
                            eo[:, :tsz], ps[:, :tsz], wr_sb[s][:, t0:t0 + tsz]
                        )
                        nc.sync.dma_start(
                            ye_d[s][d * 128:(d + 1) * 128, t0:t0 + tsz], eo[:, :tsz]
                        )

            # ---- phase D: shared GEMM1 over all T tokens ----
            for tcn in range(4):
                xts = xtpool.tile([128, KD, 512], _DT)
                nc.sync.dma_start(xts[:], xt_d[tcn])
                for hs in range(KHS):
                    ps = pspool.tile([128, 512], mybir.dt.float32)
                    for k in range(KD):
                        nc.tensor.matmul(
                            ps[:],
                            ws1_sb[:, k, hs * 128:(hs + 1) * 128],
                            xts[:, k, :],
                            start=(k == 0),
                            stop=(k == KD - 1),
                        )
                    nc.scalar.activation(
                        hsT[:, hs, tcn * 512:(tcn + 1) * 512], ps[:], gelu,
                        bias=bs1_sb[:, hs:hs + 1],
                    )

            # ---- phase E: shared GEMM2 ----
            for t in range(T // 128):
                for dh in range(2):
                    ps = pspool.tile([128, 512], mybir.dt.float32)
                    for k in range(KHS):
                        nc.tensor.matmul(
                            ps[:],
                            hsT[:, k, t * 128:(t + 1) * 128],
                            ws2_sb[:, k, dh * 512:(dh + 1) * 512],
                            start=(k == 0),
                            stop=(k == KHS - 1),
                        )
                    so = opool.tile([128, 512], _DT, tag="so")
                    nc.vector.tensor_copy(so[:], ps[:])
                    nc.sync.dma_start(
                        sh_d[t * 128:(t + 1) * 128, dh * 512:(dh + 1) * 512], so[:]
                    )

    nc.finalize()
    return nc


def _routing(xf, Wg, bg, bias):
    """Host gate: fp64 for a stable top-2 ranking (matches fp32 reference
    ordering except for ~1e-7-wide ties, which don't occur at these margins)."""
    logits = xf.astype(np.float64) @ Wg.T.astype(np.float64) + bg + bias
    scores = (1.0 / (1.0 + np.exp(-logits))).astype(np.float32)
    # stable sort => ties break toward the lower expert index, like lax.top_k
    top_idx = np.argsort(-scores, axis=1, kind="stable")[:, :TOP_K]
    top_w = np.take_along_axis(scores, top_idx, axis=1)
    return top_idx, top_w


def _round64(n):
    return max(64, -(-n // 64) * 64)


def kernel(x, Wg, bg, bias, W1, b1, W2, b2, Ws1, bs1, Ws2, bs2):
    x = np.asarray(x, F32)
    Wg, bg, bias = np.asarray(Wg, F32), np.asarray(bg, F32), np.asarray(bias, F32)
    W1, b1 = np.asarray(W1, F32), np.asarray(b1, F32)
    W2, b2 = np.asarray(W2, F32), np.asarray(b2, F32)
    Ws1, bs1 = np.asarray(Ws1, F32), np.asarray(bs1, F32)
    Ws2, bs2 = np.asarray(Ws2, F32), np.asarray(bs2, F32)

    xf = x.reshape(-1, D)
    top_idx, top_w = _routing(xf, Wg, bg, bias)

    sels, ws = [], []
    for e in range(E):
        pick = (top_idx == e)
        sel = np.where(pick.any(axis=1))[0]
        w = np.where(pick[sel, 0], top_w[sel, 0], top_w[sel, 1]).astype(F32)
        sels.append(sel)
        ws.append(w)
    counts = np.array([len(s) for s in sels])
    order = np.argsort(-counts, kind="stable")
    heavy, light = order[:4], order[4:]
    caps = (_round64(counts[heavy].max()), _round64(counts[light].max()))

    if caps not in _cache:
        _cache[caps] = _build(caps)
    nc = _cache[caps]

    x_bf = xf.astype(BF16)
    # xt: [4, 128, KD, 512]  (token-chunk major, partition-major inside)
    xt = np.ascontiguousarray(
        x_bf.T.reshape(KD, 128, 4, 512).transpose(2, 1, 0, 3)
    )

    # Half-expert weight re-layouts are input-independent; cache across calls
    # (keyed by content hash, so a reused buffer can't serve stale layouts).
    hsh = hashlib.blake2b(digest_size=16)
    for a in (W1, W2, Ws1, Ws2, b1, bs1):
        hsh.update(np.ascontiguousarray(a).data)
    wkey = hsh.hexdigest()
    wmaps = _wcache.get(wkey)
    if wmaps is None:
        wmaps = {"half": {}, "core": []}
        for e in range(E):
            for hf in range(2):
                r0 = hf * H2
                wmaps["half"][(e, hf)] = {
                    # W1 half rows -> W1halfT [D, H2] -> [8, 128, KD, 256]
                    "w1": np.ascontiguousarray(
                        W1[e][r0:r0 + H2].T.reshape(KD, 128, KH2 // 2, 256)
                        .transpose(2, 1, 0, 3).astype(BF16)
                    ),
                    # W2 half cols -> W2halfT [H2, D] -> [8, 128, KH2, 128]
                    "w2": np.ascontiguousarray(
                        W2[e][:, r0:r0 + H2].T.reshape(KH2, 128, 8, 128)
                        .transpose(2, 1, 0, 3).astype(BF16)
                    ),
                    "b1": np.ascontiguousarray(b1[e][r0:r0 + H2].reshape(KH2, 128).T),
                }
        for c in range(N_CORES):
            hs0 = c * HS
            wmaps["core"].append({
                "ws1": np.ascontiguousarray(
                    Ws1[hs0:hs0 + HS].T.reshape(KD, 128, HS)
                    .transpose(1, 0, 2).astype(BF16)
                ),
                "ws2": np.ascontiguousarray(
                    Ws2[:, hs0:hs0 + HS].T.reshape(KHS, 128, D)
                    .transpose(1, 0, 2).astype(BF16)
                ),
                "bs1c": np.ascontiguousarray(bs1[hs0:hs0 + HS].reshape(KHS, 128).T),
            })
        _wcache.clear()
        _wcache[wkey] = wmaps

    # per-expert gathered tokens + combine weights at the slot cap
    def gathered(e, cap):
        sel, w = sels[e], ws[e]
        xe = np.zeros((cap, D), BF16)
        xe[: len(sel)] = x_bf[sel]
        xe_t = np.ascontiguousarray(xe.T.reshape(KD, 128, cap).transpose(1, 0, 2))
        wpad = np.zeros(cap, F32)
        wpad[: len(w)] = w
        wr = np.ascontiguousarray(np.broadcast_to(wpad, (128, cap)))
        return xe_t, wr

    gcache = {}
    in_maps = []
    for c in range(N_CORES):
        m = {"xt": xt, **wmaps["core"][c]}
        for s, grp, cap in (("a", heavy, caps[0]), ("b", light, caps[1])):
            e, hf = int(grp[c // 2]), c % 2
            if e not in gcache:
                gcache[e] = gathered(e, cap)
            m[f"xe{s}"], m[f"wr{s}"] = gcache[e]
            half = wmaps["half"][(e, hf)]
            m[f"w1{s}"], m[f"w2{s}"], m[f"b1{s}"] = half["w1"], half["w2"], half["b1"]
        in_maps.append(m)

    res = run_bass_kernel_spmd(nc, in_maps, core_ids=list(range(N_CORES)))

    out = np.zeros((T, D), F32)
    for c in range(N_CORES):
        out += res.results[c]["sh"].astype(F32)
        for s, grp in (("a", heavy), ("b", light)):
            e = int(grp[c // 2])
            sel = sels[e]
            out[sel] += res.results[c][f"ye{s}"][:, : len(sel)].T
    # biases handled host-side: per-token weighted b2, plus bs2
    wdense = np.zeros((T, E), F32)
    np.put_along_axis(wdense, top_idx, top_w, axis=1)
    out += wdense @ b2
    out += bs2
    return out.reshape(x.shape)
